# revision 33
# baseline (speedup 1.0000x reference)
"""KWS-SNN Trainium2 kernel: 8-way batch-parallel Bass/Tile implementation.

Per core (BC=64): mean over T (DMA-fed vector adds) -> padded image to DRAM
-> conv1 (block-diag batch-packed, K=72, fp32r) with single-op 2x2 PSUM
pool-reduce -> BN+ReLU into SBUF-resident padded f1 -> conv2 (9 tap-matmuls,
K=64, fp32r) pool-reduce -> BN+ReLU -> featd round trip -> fc1 GEMM against
SBUF-preloaded weights (fp32r) -> transpose -> 25-step LIF scan -> spikes.
DMA is spread across the two HWDGE queues (sync/scalar) + gpsimd SWDGE.
"""
import sys
sys.path.insert(0, '/opt/trn_rl_repo')
import numpy as np
import concourse.bass as bass
import concourse.mybir as mybir
import bass_rust
from concourse.tile import TileContext
from concourse import bass_utils

F32 = mybir.dt.float32
F32R = mybir.dt.float32r
AL = mybir.AluOpType
AF = mybir.ActivationFunctionType
AX = mybir.AxisListType

T, BF, H, W = 25, 512, 100, 64
NCORE = 8
BC = BF // NCORE          # 64 batches per core
NCL = 35

XMP_B = 102 * 66          # 6732 per-batch padded mean image


def rap(handle, off, dims, dt=None):
    a = handle.ap()
    if dt is not None:
        a = a.bitcast(dt)
    return bass_rust.AP(tensor=a.tensor, offset=off, ap=[list(d) for d in dims])


def split_multi_waits(nc, max_waits=1):
    """This walrus build rejects >1 sync-wait per instruction; hoist excess
    waits onto same-engine NoOps inserted immediately before."""
    ctr = 0
    for f in nc.m.functions:
        for bb in f.blocks:
            il = bb.instructions
            if not any(i.sync_info is not None and len(i.sync_info.on_wait) > max_waits
                       for i in il):
                continue
            new = []
            for inst in il:
                si = inst.sync_info
                if si is not None and len(si.on_wait) > max_waits:
                    waits = list(si.on_wait)
                    while len(waits) > max_waits:
                        w = waits.pop(0)
                        nop = mybir.InstNoOp(
                            name=f"_ws_{ctr}", engine=inst.engine,
                            sync_info=mybir.SyncInfo(on_wait=[w], on_update=[]),
                            bass_nofuse=True)
                        ctr += 1
                        new.append(nop)
                    inst.sync_info = mybir.SyncInfo(
                        on_wait=waits, on_update=list(si.on_update))
                new.append(inst)
            bb.instructions = new
    return ctr


def build(be1, be2, be3):
    nc = bass.Bass()
    xin = nc.dram_tensor("x", [T, BC, H, W], F32, kind="ExternalInput")
    w1b = nc.dram_tensor("w1b", [72, 128], F32, kind="ExternalInput")
    w2t = nc.dram_tensor("w2t", [9, 64, 128], F32, kind="ExternalInput")
    w1t = nc.dram_tensor("w1t", [12800, 256], F32, kind="ExternalInput")
    cst = nc.dram_tensor("cst", [128, 8], F32, kind="ExternalInput")
    w2a = nc.dram_tensor("w2a", [128, 128], F32, kind="ExternalInput")
    w2b = nc.dram_tensor("w2b", [128, 128], F32, kind="ExternalInput")
    w3t = nc.dram_tensor("w3t", [128, 35], F32, kind="ExternalInput")
    ident = nc.dram_tensor("ident", [64, 64], F32, kind="ExternalInput")

    xmp = nc.dram_tensor("xmp", [BC * XMP_B], F32, kind="Internal")
    out = nc.dram_tensor("out", [T, BC, NCL], F32, kind="ExternalOutput")
    DBG = bool(__import__("os").environ.get("SNN_DBG"))
    if DBG:
        dfeat = nc.dram_tensor("dfeat", [BC, 12800], F32, kind="ExternalOutput")
        dcur = nc.dram_tensor("dcur", [128, 128], F32, kind="ExternalOutput")
        dxmp = nc.dram_tensor("dxmp", [BC * XMP_B], F32, kind="ExternalOutput")
        dhp = nc.dram_tensor("dhp", [128, 1600], F32, kind="ExternalOutput")
        df1 = nc.dram_tensor("df1", [128, 1768], F32, kind="ExternalOutput")

    with TileContext(nc) as tc:
        with (
            tc.tile_pool(name="const", bufs=1) as pc,
            tc.tile_pool(name="main", bufs=1) as pm,
            tc.tile_pool(name="psA", bufs=3, space="PSUM") as psA,
            tc.tile_pool(name="psB", bufs=3, space="PSUM") as psB,
        ):
            # ---- constants to SBUF ----
            w1b_sb = pc.tile([72, 128], F32R, tag="w1b")
            nc.sync.dma_start(w1b_sb[:], w1b.ap().bitcast(F32R))
            # conv2 weights duplicated into both partition halves so either
            # f1pad 64-slice can be the matmul rhs (equal base-partition rule)
            w2t_sb = pc.tile([128, 9 * 128], F32R, tag="w2t")
            nc.sync.dma_start(
                w2t_sb[0:64, :], rap(w2t, 0, [[128, 64], [8192, 9], [1, 128]], F32R))
            nc.sync.dma_start(
                w2t_sb[64:128, :], rap(w2t, 0, [[128, 64], [8192, 9], [1, 128]], F32R))
            # packed per-partition constants: cols = bn1s bn1b bn2s bn2b b1h0 b1h1 b2 b3
            cst_sb = pc.tile([128, 8], F32, tag="cst")
            nc.scalar.dma_start(cst_sb[:], cst.ap())
            bn1s_sb, bn1b_sb = cst_sb[:, 0:1], cst_sb[:, 1:2]
            bn2s_sb, bn2b_sb = cst_sb[:, 2:3], cst_sb[:, 3:4]
            b1_sb = cst_sb[:, 4:6]
            b2_sb = cst_sb[:, 6:7]
            b3_sb = cst_sb[0:35, 7:8]
            w2a_sb = pc.tile([128, 128], F32R, tag="w2a")
            nc.scalar.dma_start(w2a_sb[:], w2a.ap().bitcast(F32R))
            w2b_sb = pc.tile([128, 128], F32R, tag="w2b")
            nc.scalar.dma_start(w2b_sb[:], w2b.ap().bitcast(F32R))
            w3t_sb = pc.tile([128, 35], F32R, tag="w3t")
            nc.scalar.dma_start(w3t_sb[:], w3t.ap().bitcast(F32R))
            id_sb = pc.tile([64, 64], F32, tag="id")
            nc.scalar.dma_start(id_sb[:], ident.ap())

            cur1T = pm.tile([128, 128], F32, tag="cur1T")
            featsb = pm.tile([64, 12800], F32, tag="featsb")

            # ---- phase A: sum over T (mean /25 folded into conv1 w) ----
            with (
                tc.tile_pool(name="phZ", bufs=1) as pz,
                tc.tile_pool(name="phA", bufs=4) as pa,
            ):
                # zero-fill padded image surface early
                zt = pz.tile([128, 3366], F32, tag="zt")
                nc.gpsimd.memset(zt[:], 0.0)
                nc.sync.dma_start(
                    rap(xmp, 0, [[3366, 128], [1, 3366]]), zt[:])
                acc = pz.tile([128, 3200], F32, tag="acc")
                for t in range(T):
                    xt = pa.tile([128, 3200], F32, tag="xt")
                    eng = nc.sync if t % 2 == 0 else nc.scalar
                    # partition = half*64 + batch so each image half is a
                    # contiguous partition range for the padded scatter below
                    eng.dma_start(
                        xt[:], rap(xin, t * 409600,
                                   [[3200, 2], [6400, 64], [1, 3200]]))
                    if t == 0:
                        nc.vector.tensor_copy(acc[:], xt[:])
                    else:
                        nc.vector.tensor_add(acc[:], acc[:], xt[:])
                # scatter sum into padded per-batch images (one DMA per half)
                for h in range(2):
                    nc.sync.dma_start(
                        rap(xmp, 67 + h * 3300, [[XMP_B, 64], [66, 50], [1, 64]]),
                        acc[h * 64:(h + 1) * 64, :].rearrange(
                            "p (r c) -> p r c", c=64))

            # ---- conv1 + conv2, pipelined per 8-batch chunk ----
            with (
                tc.tile_pool(name="phC", bufs=2) as p1,
                tc.tile_pool(name="phD", bufs=2) as p2,
                tc.tile_pool(name="phF1", bufs=2) as pf,
            ):
                rblk = [(0, 14), (14, 12), (26, 12), (38, 12)]
                for c in range(8):
                    im1 = p1.tile([72, 6400], F32R, tag="im1")
                    for dh in range(3):
                        for dw in range(3):
                            eng = nc.sync if (dh * 3 + dw) % 2 == 0 else nc.scalar
                            eng.dma_start(
                                im1[dh * 24 + dw * 8:dh * 24 + dw * 8 + 8, :],
                                rap(xmp, c * 8 * XMP_B + dh * 66 + dw,
                                    [[XMP_B, 8], [66, 100], [1, 64]], F32R))
                    f1pad = pf.tile([128, 1768], F32R, tag="f1pad")
                    nc.gpsimd.memset(f1pad[:].bitcast(F32), 0.0)
                    hp = p1.tile([128, 1600], F32, tag="hp")
                    for s in range(13):
                        n = 512 if s < 12 else 256
                        rows = 8 if s < 12 else 4
                        ps = psA.tile([128, 512], F32, tag="cv")
                        nc.tensor.matmul(
                            ps[:, 0:n], w1b_sb[:],
                            im1[:, s * 512:s * 512 + n],
                            start=True, stop=True)
                        pv = ps[:, 0:n].rearrange(
                            "p (ro t1 wo t2) -> p ro wo t1 t2",
                            t1=2, wo=32, t2=2)
                        nc.vector.tensor_reduce(
                            hp[:, s * 128:s * 128 + rows * 16].rearrange(
                                "p (ro wo) -> p ro wo", wo=32),
                            pv, AX.XY, AL.max)
                    fv = f1pad[:].rearrange("p (r w) -> p r w", w=34)
                    nc.scalar.activation(
                        fv[:, 1:51, 1:33],
                        hp[:].rearrange("p (r w) -> p r w", w=32),
                        AF.Relu, bias=bn1b_sb, scale=bn1s_sb)
                    if DBG and c == 7:
                        nc.sync.dma_start(dhp.ap(), hp[:])
                        nc.sync.dma_start(df1.ap(), f1pad[:].bitcast(F32))

                    for g2 in range(2):
                        g = c * 2 + g2
                        fqv = f1pad[g2 * 64:(g2 + 1) * 64, :].rearrange(
                            "p (r w) -> p r w", w=34)
                        p2f = p2.tile([128, 400], F32, tag="p2f")
                        for (r0, nr) in rblk:
                            ps = psA.tile([128, 512], F32, tag="cv")
                            n = nr * 32
                            for ti in range(9):
                                dh, dw = ti // 3, ti % 3
                                nc.tensor.matmul(
                                    ps[:, 0:n],
                                    w2t_sb[g2 * 64:(g2 + 1) * 64,
                                           ti * 128:(ti + 1) * 128],
                                    fqv[:, dh + r0:dh + r0 + nr, dw:dw + 32],
                                    start=(ti == 0), stop=(ti == 8))
                            pv = ps[:, 0:n].rearrange(
                                "p (ro t1 wo t2) -> p ro wo t1 t2",
                                t1=2, wo=16, t2=2)
                            nc.vector.tensor_reduce(
                                p2f[:, (r0 // 2) * 16:((r0 + nr) // 2) * 16]
                                .rearrange("p (ro wo) -> p ro wo", wo=16),
                                pv, AX.XY, AL.max)
                        p2a = p2.tile([128, 400], F32, tag="p2a")
                        nc.scalar.activation(p2a[:], p2f[:], AF.Relu,
                                             bias=bn2b_sb,
                                             scale=bn2s_sb)
                        for bq in range(4):
                            eng = nc.sync if bq % 2 == 0 else nc.scalar
                            eng.dma_start(
                                featsb[g * 4 + bq:g * 4 + bq + 1, :].rearrange(
                                    "p (c q) -> p c q", q=400),
                                p2a[bq * 32:(bq + 1) * 32, :])

            # ---- fc1 GEMM: K=12800; lhsT tiles via PE transpose of featsb ----
            with tc.tile_pool(name="phE", bufs=3) as p4:
                psf = psB.tile([64, 256], F32, tag="b")
                for kk in range(13):
                    nt = 8 if kk < 12 else 4
                    wt8 = p4.tile([128, 8 * 256], F32R, tag="wt8")
                    eng = nc.sync if kk % 2 == 0 else nc.scalar
                    eng.dma_start(
                        wt8[:, 0:nt * 256],
                        rap(w1t, kk * 8 * 32768,
                            [[256, 128], [32768, nt], [1, 256]], F32R))
                    for j in range(nt):
                        k = kk * 8 + j
                        ftp = psB.tile([128, 64], F32, tag="ftp", bufs=2)
                        nc.tensor.transpose(
                            ftp[:], featsb[:, k * 128:(k + 1) * 128], id_sb[:])
                        ft = p4.tile([128, 64], F32R, tag="ft", bufs=4)
                        nc.scalar.activation(ft[:], ftp[:], AF.Copy,
                                             bias=0.0, scale=1.0)
                        nc.tensor.matmul(psf[:], ft[:],
                                         wt8[:, j * 256:(j + 1) * 256],
                                         start=(k == 0), stop=(k == 99))
                cur1 = p4.tile([64, 256], F32, tag="cur1")
                nc.scalar.copy(cur1[:], psf[:])
                for h in range(2):
                    pst = psB.tile([128, 64], F32, tag="b")
                    nc.tensor.transpose(pst[:], cur1[:, h * 128:(h + 1) * 128],
                                        id_sb[:])
                    nc.vector.tensor_scalar(cur1T[:, h * 64:(h + 1) * 64],
                                            pst[:], b1_sb[:, h:h + 1], None,
                                            AL.add)

            if DBG:
                nc.sync.dma_start(dfeat.ap(), featsb[:])
                nc.sync.dma_start(dcur.ap(), cur1T[:])
                nc.sync.dma_start(dxmp.ap(), xmp.ap())

            # ---- LIF scan (reset_t == spike_{t-1}; biases on scalar engine) ----
            with tc.tile_pool(name="phF", bufs=3) as p5, \
                 tc.tile_pool(name="phG", bufs=1) as p6:
                outsb = p6.tile([35, T * 64], F32, tag="outsb")
                m1 = p6.tile([128, 128], F32, tag="m1")
                m2 = p6.tile([128, 64], F32, tag="m2")
                m3 = p6.tile([35, 64], F32, tag="m3")
                s1z = p6.tile([128, 128], F32R, tag="s1z")
                s2z = p6.tile([128, 64], F32R, tag="s2z")
                nc.gpsimd.memset(m1[:], 0.0)
                nc.gpsimd.memset(m2[:], 0.0)
                nc.gpsimd.memset(m3[:], 0.0)
                nc.gpsimd.memset(s1z[:].bitcast(F32), 0.0)
                nc.gpsimd.memset(s2z[:].bitcast(F32), 0.0)
                s1_prev, s2_prev = s1z, s2z
                for t in range(T):
                    # layer 1: m1 = be1*m1 + (cur1T - s1_prev); s1 = m1>1
                    t1 = p5.tile([128, 128], F32, tag="t1")
                    nc.vector.scalar_tensor_tensor(
                        t1[:], s1_prev[:].bitcast(F32), -1.0, cur1T[:],
                        AL.mult, AL.add)
                    nc.vector.scalar_tensor_tensor(
                        m1[:], m1[:], be1, t1[:], AL.mult, AL.add)
                    s1 = p5.tile([128, 128], F32R, tag="s1")
                    nc.vector.tensor_scalar(s1[:], m1[:], 1.0, None, AL.is_gt)
                    ps2 = psB.tile([128, 64], F32, tag="b")
                    nc.tensor.matmul(ps2[:], w2a_sb[:],
                                     s1[:, 0:64],
                                     start=True, stop=False)
                    nc.tensor.matmul(ps2[:], w2b_sb[:],
                                     s1[:, 64:128],
                                     start=False, stop=True)
                    # layer 2: in2 = ps2 + b2 - s2_prev (scalar engine adds bias)
                    t2 = p5.tile([128, 64], F32, tag="t2")
                    nc.scalar.activation(t2[:], ps2[:], AF.Identity,
                                         bias=b2_sb, scale=1.0)
                    nc.vector.scalar_tensor_tensor(
                        t2[:], s2_prev[:].bitcast(F32), -1.0, t2[:],
                        AL.mult, AL.add)
                    nc.vector.scalar_tensor_tensor(
                        m2[:], m2[:], be2, t2[:], AL.mult, AL.add)
                    s2 = p5.tile([128, 64], F32R, tag="s2")
                    nc.vector.tensor_scalar(s2[:], m2[:], 1.0, None, AL.is_gt)
                    ps3 = psB.tile([35, 64], F32, tag="b")
                    nc.tensor.matmul(ps3[:], w3t_sb[:],
                                     s2[:],
                                     start=True, stop=True)
                    # layer 3: m3 = be3*m3 + (ps3 + b3 - s3_prev)
                    t3 = p5.tile([35, 64], F32, tag="t3")
                    nc.scalar.activation(t3[:], ps3[:], AF.Identity,
                                         bias=b3_sb, scale=1.0)
                    s3prev = outsb[:, (t - 1) * 64:t * 64] if t > 0 else outsb[:, 0:64]
                    if t > 0:
                        nc.vector.tensor_sub(t3[:], t3[:], s3prev)
                    nc.vector.scalar_tensor_tensor(
                        m3[:], m3[:], be3, t3[:], AL.mult, AL.add)
                    nc.vector.tensor_scalar(outsb[:, t * 64:(t + 1) * 64],
                                            m3[:], 1.0, None, AL.is_gt)
                    s1_prev, s2_prev = s1, s2
                # transpose spikes to (t,b)-major so the out DMA writes
                # 140B-contiguous lines instead of a 4-byte scatter
                osb2 = p6.tile([128, 13 * 35], F32, tag="osb2")
                for cchunk in range(13):
                    n = 128 if cchunk < 12 else 64
                    pso = psB.tile([128, 35], F32, tag="ftp", bufs=2)
                    nc.tensor.transpose(
                        pso[0:n, :], outsb[:, cchunk * 128:cchunk * 128 + n],
                        id_sb[0:35, 0:35])
                    nc.vector.tensor_copy(
                        osb2[0:n, cchunk * 35:(cchunk + 1) * 35], pso[0:n, :])
                nc.sync.dma_start(
                    rap(out, 0, [[35, 128], [4480, 12], [1, 35]]),
                    osb2[:, 0:12 * 35].rearrange("p (c q) -> p c q", q=35))
                nc.sync.dma_start(
                    rap(out, 12 * 4480, [[35, 64], [1, 35]]),
                    osb2[0:64, 12 * 35:13 * 35])

    split_multi_waits(nc)
    return nc


def prep(inputs):
    f = np.float32
    w1 = np.asarray(inputs["conv1_w"], f)
    s1v = np.asarray(inputs["bn1_g"], f) / np.sqrt(
        np.asarray(inputs["bn1_v"], f) + 1e-5)
    sh1 = np.asarray(inputs["bn1_b"], f) + (
        np.asarray(inputs["conv1_b"], f) - np.asarray(inputs["bn1_m"], f)) * s1v
    w2 = np.asarray(inputs["conv2_w"], f)
    s2v = np.asarray(inputs["bn2_g"], f) / np.sqrt(
        np.asarray(inputs["bn2_v"], f) + 1e-5)
    sh2 = np.asarray(inputs["bn2_b"], f) + (
        np.asarray(inputs["conv2_b"], f) - np.asarray(inputs["bn2_m"], f)) * s2v

    w1b = np.zeros((72, 128), f)
    for bg in range(8):
        for ch in range(16):
            for dh in range(3):
                for dw in range(3):
                    w1b[dh * 24 + dw * 8 + bg, bg * 16 + ch] = \
                        w1[ch, 0, dh, dw] / 25.0
    bn1sv = np.tile(s1v, 8).astype(f)
    bn1bv = np.tile(sh1, 8).astype(f)

    w2t9 = np.zeros((9, 64, 128), f)
    for ti in range(9):
        dh, dw = ti // 3, ti % 3
        for bg in range(4):
            for ci in range(16):
                for co in range(32):
                    w2t9[ti, bg * 16 + ci, bg * 32 + co] = w2[co, ci, dh, dw]
    bn2sv = np.tile(s2v, 4).astype(f)
    bn2bv = np.tile(sh2, 4).astype(f)

    b1v = np.asarray(inputs["fc1_b"], f)
    b2v = np.asarray(inputs["fc2_b"], f)
    b3v = np.asarray(inputs["fc3_b"], f)
    cst = np.zeros((128, 8), f)
    cst[:, 0] = bn1sv
    cst[:, 1] = bn1bv
    cst[:, 2] = bn2sv
    cst[:, 3] = bn2bv
    cst[:, 4] = b1v[0:128]
    cst[:, 5] = b1v[128:256]
    cst[:, 6] = b2v
    cst[0:35, 7] = b3v

    return dict(
        w1b=w1b, w2t=w2t9, cst=cst,
        w1t=np.ascontiguousarray(np.asarray(inputs["fc1_w"], f).T),
        w2a=np.ascontiguousarray(np.asarray(inputs["fc2_w"], f).T[0:128]),
        w2b=np.ascontiguousarray(np.asarray(inputs["fc2_w"], f).T[128:256]),
        w3t=np.ascontiguousarray(np.asarray(inputs["fc3_w"], f).T),
        ident=np.eye(64, dtype=f),
    )


def kernel(**inputs):
    f = np.float32
    x = np.asarray(inputs["x"], f)
    be1 = float(np.clip(np.asarray(inputs["beta1"], f), 0.0, 1.0))
    be2 = float(np.clip(np.asarray(inputs["beta2"], f), 0.0, 1.0))
    be3 = float(np.clip(np.asarray(inputs["beta3"], f), 0.0, 1.0))
    consts = prep(inputs)
    nc = build(be1, be2, be3)
    in_maps = []
    for c in range(NCORE):
        m = {"x": np.ascontiguousarray(x[:, c * BC:(c + 1) * BC])}
        m.update(consts)
        in_maps.append(m)
    res = bass_utils.run_bass_kernel_spmd(nc, in_maps, core_ids=list(range(NCORE)))
    return np.concatenate([res.results[c]["out"] for c in range(NCORE)], axis=1)


# revision 34
# speedup vs baseline: 1.8851x; 1.8851x over previous
"""KWS-SNN Trainium2 kernel: 8-way batch-parallel Bass/Tile implementation.

Per core (BC=64): mean over T (DMA-fed vector adds) -> padded image to DRAM
-> conv1 (block-diag batch-packed, K=72, fp32r) with single-op 2x2 PSUM
pool-reduce -> BN+ReLU into SBUF-resident padded f1 -> conv2 (9 tap-matmuls,
K=64, fp32r) pool-reduce -> BN+ReLU -> featd round trip -> fc1 GEMM against
SBUF-preloaded weights (fp32r) -> transpose -> 25-step LIF scan -> spikes.
DMA is spread across the two HWDGE queues (sync/scalar) + gpsimd SWDGE.
"""
import sys
sys.path.insert(0, '/opt/trn_rl_repo')
import numpy as np
import concourse.bass as bass
import concourse.mybir as mybir
import bass_rust
from concourse.tile import TileContext
from concourse import bass_utils

F32 = mybir.dt.float32
F32R = mybir.dt.float32r
AL = mybir.AluOpType
AF = mybir.ActivationFunctionType
AX = mybir.AxisListType

T, BF, H, W = 25, 512, 100, 64
NCORE = 8
BC = BF // NCORE          # 64 batches per core
NCL = 35

XMP_B = 102 * 66          # 6732 per-batch padded mean image


def rap(handle, off, dims, dt=None):
    a = handle.ap()
    if dt is not None:
        a = a.bitcast(dt)
    return bass_rust.AP(tensor=a.tensor, offset=off, ap=[list(d) for d in dims])


def split_multi_waits(nc, max_waits=1):
    """This walrus build rejects >1 sync-wait per instruction; hoist excess
    waits onto same-engine NoOps inserted immediately before."""
    ctr = 0
    for f in nc.m.functions:
        for bb in f.blocks:
            il = bb.instructions
            if not any(i.sync_info is not None and len(i.sync_info.on_wait) > max_waits
                       for i in il):
                continue
            new = []
            for inst in il:
                si = inst.sync_info
                if si is not None and len(si.on_wait) > max_waits:
                    waits = list(si.on_wait)
                    while len(waits) > max_waits:
                        w = waits.pop(0)
                        nop = mybir.InstNoOp(
                            name=f"_ws_{ctr}", engine=inst.engine,
                            sync_info=mybir.SyncInfo(on_wait=[w], on_update=[]),
                            bass_nofuse=True)
                        ctr += 1
                        new.append(nop)
                    inst.sync_info = mybir.SyncInfo(
                        on_wait=waits, on_update=list(si.on_update))
                new.append(inst)
            bb.instructions = new
    return ctr


def build(be1, be2, be3):
    nc = bass.Bass()
    xin = nc.dram_tensor("x", [T, BC, H, W], F32, kind="ExternalInput")
    w1b = nc.dram_tensor("w1b", [72, 128], F32, kind="ExternalInput")
    w2t = nc.dram_tensor("w2t", [9, 64, 128], F32, kind="ExternalInput")
    w1t = nc.dram_tensor("w1t", [12800, 256], F32, kind="ExternalInput")
    cst = nc.dram_tensor("cst", [128, 8], F32, kind="ExternalInput")
    w2a = nc.dram_tensor("w2a", [128, 128], F32, kind="ExternalInput")
    w2b = nc.dram_tensor("w2b", [128, 128], F32, kind="ExternalInput")
    w3t = nc.dram_tensor("w3t", [128, 35], F32, kind="ExternalInput")
    ident = nc.dram_tensor("ident", [64, 64], F32, kind="ExternalInput")

    xmp = nc.dram_tensor("xmp", [BC * XMP_B], F32, kind="Internal")
    xm = nc.dram_tensor("xm", [BC * 6400], F32, kind="Internal")
    out = nc.dram_tensor("out", [T, BC, NCL], F32, kind="ExternalOutput")
    DBG = bool(__import__("os").environ.get("SNN_DBG"))
    if DBG:
        dfeat = nc.dram_tensor("dfeat", [BC, 12800], F32, kind="ExternalOutput")
        dcur = nc.dram_tensor("dcur", [128, 128], F32, kind="ExternalOutput")
        dxmp = nc.dram_tensor("dxmp", [BC * XMP_B], F32, kind="ExternalOutput")
        dhp = nc.dram_tensor("dhp", [128, 1600], F32, kind="ExternalOutput")
        df1 = nc.dram_tensor("df1", [128, 1768], F32, kind="ExternalOutput")

    with TileContext(nc) as tc:
        with (
            tc.tile_pool(name="const", bufs=1) as pc,
            tc.tile_pool(name="main", bufs=1) as pm,
            tc.tile_pool(name="psA", bufs=3, space="PSUM") as psA,
            tc.tile_pool(name="psB", bufs=3, space="PSUM") as psB,
        ):
            # ---- constants to SBUF ----
            w1b_sb = pc.tile([72, 128], F32R, tag="w1b")
            nc.sync.dma_start(w1b_sb[:], w1b.ap().bitcast(F32R))
            # conv2 weights duplicated into both partition halves so either
            # f1pad 64-slice can be the matmul rhs (equal base-partition rule)
            w2t_sb = pc.tile([128, 9 * 128], F32R, tag="w2t")
            nc.sync.dma_start(
                w2t_sb[0:64, :], rap(w2t, 0, [[128, 64], [8192, 9], [1, 128]], F32R))
            nc.sync.dma_start(
                w2t_sb[64:128, :], rap(w2t, 0, [[128, 64], [8192, 9], [1, 128]], F32R))
            # packed per-partition constants: cols = bn1s bn1b bn2s bn2b b1h0 b1h1 b2 b3
            cst_sb = pc.tile([128, 8], F32, tag="cst")
            nc.scalar.dma_start(cst_sb[:], cst.ap())
            bn1s_sb, bn1b_sb = cst_sb[:, 0:1], cst_sb[:, 1:2]
            bn2s_sb, bn2b_sb = cst_sb[:, 2:3], cst_sb[:, 3:4]
            b1_sb = cst_sb[:, 4:6]
            b2_sb = cst_sb[:, 6:7]
            b3_sb = cst_sb[0:35, 7:8]
            w2a_sb = pc.tile([128, 128], F32R, tag="w2a")
            nc.scalar.dma_start(w2a_sb[:], w2a.ap().bitcast(F32R))
            w2b_sb = pc.tile([128, 128], F32R, tag="w2b")
            nc.scalar.dma_start(w2b_sb[:], w2b.ap().bitcast(F32R))
            w3t_sb = pc.tile([128, 35], F32R, tag="w3t")
            nc.scalar.dma_start(w3t_sb[:], w3t.ap().bitcast(F32R))
            id_sb = pc.tile([64, 64], F32, tag="id")
            nc.scalar.dma_start(id_sb[:], ident.ap())

            cur1T = pm.tile([128, 128], F32, tag="cur1T")
            featsb = pm.tile([64, 12800], F32, tag="featsb")

            # ---- phase A: sum over T (mean /25 folded into conv1 w) ----
            with (
                tc.tile_pool(name="phZ", bufs=1) as pz,
                tc.tile_pool(name="phA", bufs=4) as pa,
            ):
                # zero-fill padded image surface early
                zt = pz.tile([128, 3366], F32, tag="zt")
                nc.gpsimd.memset(zt[:], 0.0)
                nc.sync.dma_start(
                    rap(xmp, 0, [[3366, 128], [1, 3366]]), zt[:])
                acc = pz.tile([128, 3200], F32, tag="acc")
                for t in range(T):
                    xt = pa.tile([128, 3200], F32, tag="xt")
                    eng = nc.sync if t % 2 == 0 else nc.scalar
                    eng.dma_start(
                        xt[:], rap(xin, t * 409600, [[3200, 128], [1, 3200]]))
                    if t == 0:
                        nc.vector.tensor_copy(acc[:], xt[:])
                    else:
                        nc.vector.tensor_add(acc[:], acc[:], xt[:])
                # dump sum flat, then pad via DRAM->DRAM scatter
                nc.sync.dma_start(
                    rap(xm, 0, [[3200, 128], [1, 3200]]), acc[:])
                nc.sync.dma_start(
                    rap(xmp, 67, [[XMP_B, 64], [66, 100], [1, 64]]),
                    rap(xm, 0, [[1, BC * 6400]]))

            # ---- conv1 + conv2, pipelined per 8-batch chunk ----
            with (
                tc.tile_pool(name="phC", bufs=2) as p1,
                tc.tile_pool(name="phD", bufs=2) as p2,
                tc.tile_pool(name="phF1", bufs=2) as pf,
            ):
                rblk = [(0, 14), (14, 12), (26, 12), (38, 12)]
                for c in range(8):
                    im1 = p1.tile([72, 6400], F32R, tag="im1")
                    for dh in range(3):
                        for dw in range(3):
                            eng = nc.sync if (dh * 3 + dw) % 2 == 0 else nc.scalar
                            eng.dma_start(
                                im1[dh * 24 + dw * 8:dh * 24 + dw * 8 + 8, :],
                                rap(xmp, c * 8 * XMP_B + dh * 66 + dw,
                                    [[XMP_B, 8], [66, 100], [1, 64]], F32R))
                    f1pad = pf.tile([128, 1768], F32R, tag="f1pad")
                    nc.gpsimd.memset(f1pad[:].bitcast(F32), 0.0)
                    hp = p1.tile([128, 1600], F32, tag="hp")
                    for s in range(13):
                        n = 512 if s < 12 else 256
                        rows = 8 if s < 12 else 4
                        ps = psA.tile([128, 512], F32, tag="cv")
                        nc.tensor.matmul(
                            ps[:, 0:n], w1b_sb[:],
                            im1[:, s * 512:s * 512 + n],
                            start=True, stop=True)
                        pv = ps[:, 0:n].rearrange(
                            "p (ro t1 wo t2) -> p ro wo t1 t2",
                            t1=2, wo=32, t2=2)
                        nc.vector.tensor_reduce(
                            hp[:, s * 128:s * 128 + rows * 16].rearrange(
                                "p (ro wo) -> p ro wo", wo=32),
                            pv, AX.XY, AL.max)
                    fv = f1pad[:].rearrange("p (r w) -> p r w", w=34)
                    nc.scalar.activation(
                        fv[:, 1:51, 1:33],
                        hp[:].rearrange("p (r w) -> p r w", w=32),
                        AF.Relu, bias=bn1b_sb, scale=bn1s_sb)
                    if DBG and c == 7:
                        nc.sync.dma_start(dhp.ap(), hp[:])
                        nc.sync.dma_start(df1.ap(), f1pad[:].bitcast(F32))

                    for g2 in range(2):
                        g = c * 2 + g2
                        fqv = f1pad[g2 * 64:(g2 + 1) * 64, :].rearrange(
                            "p (r w) -> p r w", w=34)
                        p2f = p2.tile([128, 400], F32, tag="p2f")
                        for (r0, nr) in rblk:
                            ps = psA.tile([128, 512], F32, tag="cv")
                            n = nr * 32
                            for ti in range(9):
                                dh, dw = ti // 3, ti % 3
                                nc.tensor.matmul(
                                    ps[:, 0:n],
                                    w2t_sb[g2 * 64:(g2 + 1) * 64,
                                           ti * 128:(ti + 1) * 128],
                                    fqv[:, dh + r0:dh + r0 + nr, dw:dw + 32],
                                    start=(ti == 0), stop=(ti == 8))
                            pv = ps[:, 0:n].rearrange(
                                "p (ro t1 wo t2) -> p ro wo t1 t2",
                                t1=2, wo=16, t2=2)
                            nc.vector.tensor_reduce(
                                p2f[:, (r0 // 2) * 16:((r0 + nr) // 2) * 16]
                                .rearrange("p (ro wo) -> p ro wo", wo=16),
                                pv, AX.XY, AL.max)
                        p2a = p2.tile([128, 400], F32, tag="p2a")
                        nc.scalar.activation(p2a[:], p2f[:], AF.Relu,
                                             bias=bn2b_sb,
                                             scale=bn2s_sb)
                        for bq in range(4):
                            eng = nc.sync if bq % 2 == 0 else nc.scalar
                            eng.dma_start(
                                featsb[g * 4 + bq:g * 4 + bq + 1, :].rearrange(
                                    "p (c q) -> p c q", q=400),
                                p2a[bq * 32:(bq + 1) * 32, :])

            # ---- fc1 GEMM: K=12800; lhsT tiles via PE transpose of featsb ----
            with tc.tile_pool(name="phE", bufs=3) as p4:
                psf = psB.tile([64, 256], F32, tag="b")
                for kk in range(13):
                    nt = 8 if kk < 12 else 4
                    wt8 = p4.tile([128, 8 * 256], F32R, tag="wt8")
                    eng = nc.sync if kk % 2 == 0 else nc.scalar
                    eng.dma_start(
                        wt8[:, 0:nt * 256],
                        rap(w1t, kk * 8 * 32768,
                            [[256, 128], [32768, nt], [1, 256]], F32R))
                    for j in range(nt):
                        k = kk * 8 + j
                        ftp = psB.tile([128, 64], F32, tag="ftp", bufs=2)
                        nc.tensor.transpose(
                            ftp[:], featsb[:, k * 128:(k + 1) * 128], id_sb[:])
                        ft = p4.tile([128, 64], F32R, tag="ft", bufs=4)
                        nc.scalar.activation(ft[:], ftp[:], AF.Copy,
                                             bias=0.0, scale=1.0)
                        nc.tensor.matmul(psf[:], ft[:],
                                         wt8[:, j * 256:(j + 1) * 256],
                                         start=(k == 0), stop=(k == 99))
                cur1 = p4.tile([64, 256], F32, tag="cur1")
                nc.scalar.copy(cur1[:], psf[:])
                for h in range(2):
                    pst = psB.tile([128, 64], F32, tag="b")
                    nc.tensor.transpose(pst[:], cur1[:, h * 128:(h + 1) * 128],
                                        id_sb[:])
                    nc.vector.tensor_scalar(cur1T[:, h * 64:(h + 1) * 64],
                                            pst[:], b1_sb[:, h:h + 1], None,
                                            AL.add)

            if DBG:
                nc.sync.dma_start(dfeat.ap(), featsb[:])
                nc.sync.dma_start(dcur.ap(), cur1T[:])
                nc.sync.dma_start(dxmp.ap(), xmp.ap())

            # ---- LIF scan (reset_t == spike_{t-1}; biases on scalar engine) ----
            with tc.tile_pool(name="phF", bufs=3) as p5, \
                 tc.tile_pool(name="phG", bufs=1) as p6:
                outsb = p6.tile([35, T * 64], F32, tag="outsb")
                m1 = p6.tile([128, 128], F32, tag="m1")
                m2 = p6.tile([128, 64], F32, tag="m2")
                m3 = p6.tile([35, 64], F32, tag="m3")
                s1z = p6.tile([128, 128], F32R, tag="s1z")
                s2z = p6.tile([128, 64], F32R, tag="s2z")
                nc.gpsimd.memset(m1[:], 0.0)
                nc.gpsimd.memset(m2[:], 0.0)
                nc.gpsimd.memset(m3[:], 0.0)
                nc.gpsimd.memset(s1z[:].bitcast(F32), 0.0)
                nc.gpsimd.memset(s2z[:].bitcast(F32), 0.0)
                s1_prev, s2_prev = s1z, s2z
                for t in range(T):
                    # layer 1: m1 = be1*m1 + (cur1T - s1_prev); s1 = m1>1
                    t1 = p5.tile([128, 128], F32, tag="t1")
                    nc.vector.scalar_tensor_tensor(
                        t1[:], s1_prev[:].bitcast(F32), -1.0, cur1T[:],
                        AL.mult, AL.add)
                    nc.vector.scalar_tensor_tensor(
                        m1[:], m1[:], be1, t1[:], AL.mult, AL.add)
                    s1 = p5.tile([128, 128], F32R, tag="s1")
                    nc.vector.tensor_scalar(s1[:], m1[:], 1.0, None, AL.is_gt)
                    ps2 = psB.tile([128, 64], F32, tag="b")
                    nc.tensor.matmul(ps2[:], w2a_sb[:],
                                     s1[:, 0:64],
                                     start=True, stop=False)
                    nc.tensor.matmul(ps2[:], w2b_sb[:],
                                     s1[:, 64:128],
                                     start=False, stop=True)
                    # layer 2: in2 = ps2 + b2 - s2_prev (scalar engine adds bias)
                    t2 = p5.tile([128, 64], F32, tag="t2")
                    nc.scalar.activation(t2[:], ps2[:], AF.Identity,
                                         bias=b2_sb, scale=1.0)
                    nc.vector.scalar_tensor_tensor(
                        t2[:], s2_prev[:].bitcast(F32), -1.0, t2[:],
                        AL.mult, AL.add)
                    nc.vector.scalar_tensor_tensor(
                        m2[:], m2[:], be2, t2[:], AL.mult, AL.add)
                    s2 = p5.tile([128, 64], F32R, tag="s2")
                    nc.vector.tensor_scalar(s2[:], m2[:], 1.0, None, AL.is_gt)
                    ps3 = psB.tile([35, 64], F32, tag="b")
                    nc.tensor.matmul(ps3[:], w3t_sb[:],
                                     s2[:],
                                     start=True, stop=True)
                    # layer 3: m3 = be3*m3 + (ps3 + b3 - s3_prev)
                    t3 = p5.tile([35, 64], F32, tag="t3")
                    nc.scalar.activation(t3[:], ps3[:], AF.Identity,
                                         bias=b3_sb, scale=1.0)
                    s3prev = outsb[:, (t - 1) * 64:t * 64] if t > 0 else outsb[:, 0:64]
                    if t > 0:
                        nc.vector.tensor_sub(t3[:], t3[:], s3prev)
                    nc.vector.scalar_tensor_tensor(
                        m3[:], m3[:], be3, t3[:], AL.mult, AL.add)
                    nc.vector.tensor_scalar(outsb[:, t * 64:(t + 1) * 64],
                                            m3[:], 1.0, None, AL.is_gt)
                    s1_prev, s2_prev = s1, s2
                # transpose spikes to (t,b)-major so the out DMA writes
                # 140B-contiguous lines instead of a 4-byte scatter
                osb2 = p6.tile([128, 13 * 35], F32, tag="osb2")
                for cchunk in range(13):
                    n = 128 if cchunk < 12 else 64
                    pso = psB.tile([128, 35], F32, tag="ftp", bufs=2)
                    nc.tensor.transpose(
                        pso[0:n, :], outsb[:, cchunk * 128:cchunk * 128 + n],
                        id_sb[0:35, 0:35])
                    nc.vector.tensor_copy(
                        osb2[0:n, cchunk * 35:(cchunk + 1) * 35], pso[0:n, :])
                nc.sync.dma_start(
                    rap(out, 0, [[35, 128], [4480, 12], [1, 35]]),
                    osb2[:, 0:12 * 35].rearrange("p (c q) -> p c q", q=35))
                nc.sync.dma_start(
                    rap(out, 12 * 4480, [[35, 64], [1, 35]]),
                    osb2[0:64, 12 * 35:13 * 35])

    split_multi_waits(nc)
    return nc


def prep(inputs):
    f = np.float32
    w1 = np.asarray(inputs["conv1_w"], f)
    s1v = np.asarray(inputs["bn1_g"], f) / np.sqrt(
        np.asarray(inputs["bn1_v"], f) + 1e-5)
    sh1 = np.asarray(inputs["bn1_b"], f) + (
        np.asarray(inputs["conv1_b"], f) - np.asarray(inputs["bn1_m"], f)) * s1v
    w2 = np.asarray(inputs["conv2_w"], f)
    s2v = np.asarray(inputs["bn2_g"], f) / np.sqrt(
        np.asarray(inputs["bn2_v"], f) + 1e-5)
    sh2 = np.asarray(inputs["bn2_b"], f) + (
        np.asarray(inputs["conv2_b"], f) - np.asarray(inputs["bn2_m"], f)) * s2v

    w1b = np.zeros((72, 128), f)
    for bg in range(8):
        for ch in range(16):
            for dh in range(3):
                for dw in range(3):
                    w1b[dh * 24 + dw * 8 + bg, bg * 16 + ch] = \
                        w1[ch, 0, dh, dw] / 25.0
    bn1sv = np.tile(s1v, 8).astype(f)
    bn1bv = np.tile(sh1, 8).astype(f)

    w2t9 = np.zeros((9, 64, 128), f)
    for ti in range(9):
        dh, dw = ti // 3, ti % 3
        for bg in range(4):
            for ci in range(16):
                for co in range(32):
                    w2t9[ti, bg * 16 + ci, bg * 32 + co] = w2[co, ci, dh, dw]
    bn2sv = np.tile(s2v, 4).astype(f)
    bn2bv = np.tile(sh2, 4).astype(f)

    b1v = np.asarray(inputs["fc1_b"], f)
    b2v = np.asarray(inputs["fc2_b"], f)
    b3v = np.asarray(inputs["fc3_b"], f)
    cst = np.zeros((128, 8), f)
    cst[:, 0] = bn1sv
    cst[:, 1] = bn1bv
    cst[:, 2] = bn2sv
    cst[:, 3] = bn2bv
    cst[:, 4] = b1v[0:128]
    cst[:, 5] = b1v[128:256]
    cst[:, 6] = b2v
    cst[0:35, 7] = b3v

    return dict(
        w1b=w1b, w2t=w2t9, cst=cst,
        w1t=np.ascontiguousarray(np.asarray(inputs["fc1_w"], f).T),
        w2a=np.ascontiguousarray(np.asarray(inputs["fc2_w"], f).T[0:128]),
        w2b=np.ascontiguousarray(np.asarray(inputs["fc2_w"], f).T[128:256]),
        w3t=np.ascontiguousarray(np.asarray(inputs["fc3_w"], f).T),
        ident=np.eye(64, dtype=f),
    )


def kernel(**inputs):
    f = np.float32
    x = np.asarray(inputs["x"], f)
    be1 = float(np.clip(np.asarray(inputs["beta1"], f), 0.0, 1.0))
    be2 = float(np.clip(np.asarray(inputs["beta2"], f), 0.0, 1.0))
    be3 = float(np.clip(np.asarray(inputs["beta3"], f), 0.0, 1.0))
    consts = prep(inputs)
    nc = build(be1, be2, be3)
    in_maps = []
    for c in range(NCORE):
        m = {"x": np.ascontiguousarray(x[:, c * BC:(c + 1) * BC])}
        m.update(consts)
        in_maps.append(m)
    res = bass_utils.run_bass_kernel_spmd(nc, in_maps, core_ids=list(range(NCORE)))
    return np.concatenate([res.results[c]["out"] for c in range(NCORE)], axis=1)


# revision 35
# speedup vs baseline: 1.9108x; 1.0136x over previous
"""KWS-SNN Trainium2 kernel: 8-way batch-parallel Bass/Tile implementation.

Per core (BC=64): mean over T (DMA-fed vector adds) -> padded image to DRAM
-> conv1 (block-diag batch-packed, K=72, fp32r) with single-op 2x2 PSUM
pool-reduce -> BN+ReLU into SBUF-resident padded f1 -> conv2 (9 tap-matmuls,
K=64, fp32r) pool-reduce -> BN+ReLU -> featd round trip -> fc1 GEMM against
SBUF-preloaded weights (fp32r) -> transpose -> 25-step LIF scan -> spikes.
DMA is spread across the two HWDGE queues (sync/scalar) + gpsimd SWDGE.
"""
import sys
sys.path.insert(0, '/opt/trn_rl_repo')
import numpy as np
import concourse.bass as bass
import concourse.mybir as mybir
import bass_rust
from concourse.tile import TileContext
from concourse import bass_utils

F32 = mybir.dt.float32
F32R = mybir.dt.float32r
AL = mybir.AluOpType
AF = mybir.ActivationFunctionType
AX = mybir.AxisListType

T, BF, H, W = 25, 512, 100, 64
NCORE = 8
BC = BF // NCORE          # 64 batches per core
NCL = 35

XMP_B = 102 * 66          # 6732 per-batch padded mean image


def rap(handle, off, dims, dt=None):
    a = handle.ap()
    if dt is not None:
        a = a.bitcast(dt)
    return bass_rust.AP(tensor=a.tensor, offset=off, ap=[list(d) for d in dims])


def split_multi_waits(nc, max_waits=1):
    """This walrus build rejects >1 sync-wait per instruction; hoist excess
    waits onto same-engine NoOps inserted immediately before."""
    ctr = 0
    for f in nc.m.functions:
        for bb in f.blocks:
            il = bb.instructions
            if not any(i.sync_info is not None and len(i.sync_info.on_wait) > max_waits
                       for i in il):
                continue
            new = []
            for inst in il:
                si = inst.sync_info
                if si is not None and len(si.on_wait) > max_waits:
                    waits = list(si.on_wait)
                    while len(waits) > max_waits:
                        w = waits.pop(0)
                        nop = mybir.InstNoOp(
                            name=f"_ws_{ctr}", engine=inst.engine,
                            sync_info=mybir.SyncInfo(on_wait=[w], on_update=[]),
                            bass_nofuse=True)
                        ctr += 1
                        new.append(nop)
                    inst.sync_info = mybir.SyncInfo(
                        on_wait=waits, on_update=list(si.on_update))
                new.append(inst)
            bb.instructions = new
    return ctr


def build(be1, be2, be3):
    nc = bass.Bass()
    xin = nc.dram_tensor("x", [T, BC, H, W], F32, kind="ExternalInput")
    w1b = nc.dram_tensor("w1b", [72, 128], F32, kind="ExternalInput")
    w2t = nc.dram_tensor("w2t", [9, 64, 128], F32, kind="ExternalInput")
    w1t = nc.dram_tensor("w1t", [12800, 256], F32, kind="ExternalInput")
    cst = nc.dram_tensor("cst", [128, 8], F32, kind="ExternalInput")
    w2a = nc.dram_tensor("w2a", [128, 128], F32, kind="ExternalInput")
    w2b = nc.dram_tensor("w2b", [128, 128], F32, kind="ExternalInput")
    w3t = nc.dram_tensor("w3t", [128, 35], F32, kind="ExternalInput")
    ident = nc.dram_tensor("ident", [64, 64], F32, kind="ExternalInput")

    xmp = nc.dram_tensor("xmp", [BC * XMP_B], F32, kind="Internal")
    xm = nc.dram_tensor("xm", [BC * 6400], F32, kind="Internal")
    out = nc.dram_tensor("out", [T, BC, NCL], F32, kind="ExternalOutput")
    DBG = bool(__import__("os").environ.get("SNN_DBG"))
    if DBG:
        dfeat = nc.dram_tensor("dfeat", [BC, 12800], F32, kind="ExternalOutput")
        dcur = nc.dram_tensor("dcur", [128, 128], F32, kind="ExternalOutput")
        dxmp = nc.dram_tensor("dxmp", [BC * XMP_B], F32, kind="ExternalOutput")
        dhp = nc.dram_tensor("dhp", [128, 1600], F32, kind="ExternalOutput")
        df1 = nc.dram_tensor("df1", [128, 1768], F32, kind="ExternalOutput")

    with TileContext(nc) as tc:
        with (
            tc.tile_pool(name="const", bufs=1) as pc,
            tc.tile_pool(name="main", bufs=1) as pm,
            tc.tile_pool(name="psA", bufs=3, space="PSUM") as psA,
            tc.tile_pool(name="psB", bufs=3, space="PSUM") as psB,
        ):
            # ---- constants to SBUF ----
            w1b_sb = pc.tile([72, 128], F32R, tag="w1b")
            nc.sync.dma_start(w1b_sb[:], w1b.ap().bitcast(F32R))
            # conv2 weights duplicated into both partition halves so either
            # f1pad 64-slice can be the matmul rhs (equal base-partition rule)
            w2t_sb = pc.tile([128, 9 * 128], F32R, tag="w2t")
            nc.sync.dma_start(
                w2t_sb[0:64, :], rap(w2t, 0, [[128, 64], [8192, 9], [1, 128]], F32R))
            nc.sync.dma_start(
                w2t_sb[64:128, :], rap(w2t, 0, [[128, 64], [8192, 9], [1, 128]], F32R))
            # packed per-partition constants: cols = bn1s bn1b bn2s bn2b b1h0 b1h1 b2 b3
            cst_sb = pc.tile([128, 8], F32, tag="cst")
            nc.scalar.dma_start(cst_sb[:], cst.ap())
            bn1s_sb, bn1b_sb = cst_sb[:, 0:1], cst_sb[:, 1:2]
            bn2s_sb, bn2b_sb = cst_sb[:, 2:3], cst_sb[:, 3:4]
            b1_sb = cst_sb[:, 4:6]
            b2_sb = cst_sb[:, 6:7]
            b3_sb = cst_sb[0:35, 7:8]
            w2a_sb = pc.tile([128, 128], F32R, tag="w2a")
            nc.scalar.dma_start(w2a_sb[:], w2a.ap().bitcast(F32R))
            w2b_sb = pc.tile([128, 128], F32R, tag="w2b")
            nc.scalar.dma_start(w2b_sb[:], w2b.ap().bitcast(F32R))
            w3t_sb = pc.tile([128, 35], F32R, tag="w3t")
            nc.scalar.dma_start(w3t_sb[:], w3t.ap().bitcast(F32R))
            id_sb = pc.tile([64, 64], F32, tag="id")
            nc.scalar.dma_start(id_sb[:], ident.ap())

            cur1T = pm.tile([128, 128], F32, tag="cur1T")
            featsb = pm.tile([64, 12800], F32, tag="featsb")

            # ---- phase A: sum over T (mean /25 folded into conv1 w) ----
            with (
                tc.tile_pool(name="phZ", bufs=1) as pz,
                tc.tile_pool(name="phA", bufs=4) as pa,
            ):
                # zero-fill padded image surface early
                zt = pz.tile([128, 3366], F32, tag="zt")
                nc.gpsimd.memset(zt[:], 0.0)
                nc.sync.dma_start(
                    rap(xmp, 0, [[3366, 128], [1, 3366]]), zt[:])
                acc = pz.tile([128, 3200], F32, tag="acc")
                for t in range(T):
                    xt = pa.tile([128, 3200], F32, tag="xt")
                    eng = nc.sync if t % 2 == 0 else nc.scalar
                    eng.dma_start(
                        xt[:], rap(xin, t * 409600, [[3200, 128], [1, 3200]]))
                    if t == 0:
                        nc.vector.tensor_copy(acc[:], xt[:])
                    else:
                        nc.vector.tensor_add(acc[:], acc[:], xt[:])
                # dump sum flat, then pad via DRAM->DRAM scatter
                nc.sync.dma_start(
                    rap(xm, 0, [[3200, 128], [1, 3200]]), acc[:])
                nc.sync.dma_start(
                    rap(xmp, 67, [[XMP_B, 8], [66, 100], [1, 64]]),
                    rap(xm, 0, [[1, 8 * 6400]]))
                nc.scalar.dma_start(
                    rap(xmp, 8 * XMP_B + 67, [[XMP_B, 56], [66, 100], [1, 64]]),
                    rap(xm, 8 * 6400, [[1, 56 * 6400]]))

            # ---- conv1 + conv2, pipelined per 8-batch chunk ----
            with (
                tc.tile_pool(name="phC", bufs=2) as p1,
                tc.tile_pool(name="phD", bufs=2) as p2,
                tc.tile_pool(name="phF1", bufs=2) as pf,
            ):
                rblk = [(0, 14), (14, 12), (26, 12), (38, 12)]
                for c in range(8):
                    im1 = p1.tile([72, 6400], F32R, tag="im1")
                    for dh in range(3):
                        for dw in range(3):
                            eng = nc.sync if (dh * 3 + dw) % 2 == 0 else nc.scalar
                            eng.dma_start(
                                im1[dh * 24 + dw * 8:dh * 24 + dw * 8 + 8, :],
                                rap(xmp, c * 8 * XMP_B + dh * 66 + dw,
                                    [[XMP_B, 8], [66, 100], [1, 64]], F32R))
                    f1pad = pf.tile([128, 1768], F32R, tag="f1pad")
                    nc.gpsimd.memset(f1pad[:].bitcast(F32), 0.0)
                    hp = p1.tile([128, 1600], F32, tag="hp")
                    for s in range(13):
                        n = 512 if s < 12 else 256
                        rows = 8 if s < 12 else 4
                        ps = psA.tile([128, 512], F32, tag="cv")
                        nc.tensor.matmul(
                            ps[:, 0:n], w1b_sb[:],
                            im1[:, s * 512:s * 512 + n],
                            start=True, stop=True)
                        pv = ps[:, 0:n].rearrange(
                            "p (ro t1 wo t2) -> p ro wo t1 t2",
                            t1=2, wo=32, t2=2)
                        nc.vector.tensor_reduce(
                            hp[:, s * 128:s * 128 + rows * 16].rearrange(
                                "p (ro wo) -> p ro wo", wo=32),
                            pv, AX.XY, AL.max)
                    fv = f1pad[:].rearrange("p (r w) -> p r w", w=34)
                    nc.scalar.activation(
                        fv[:, 1:51, 1:33],
                        hp[:].rearrange("p (r w) -> p r w", w=32),
                        AF.Relu, bias=bn1b_sb, scale=bn1s_sb)
                    if DBG and c == 7:
                        nc.sync.dma_start(dhp.ap(), hp[:])
                        nc.sync.dma_start(df1.ap(), f1pad[:].bitcast(F32))

                    fqv0 = f1pad[0:64, :].rearrange("p (r w) -> p r w", w=34)
                    fqv1 = f1pad[64:128, :].rearrange("p (r w) -> p r w", w=34)
                    p2f0 = p2.tile([128, 400], F32, tag="p2f0")
                    p2f1 = p2.tile([128, 400], F32, tag="p2f1")
                    for (r0, nr) in rblk:
                        ps0 = psA.tile([128, 512], F32, tag="cv")
                        ps1 = psA.tile([128, 512], F32, tag="cv")
                        n = nr * 32
                        # interleave the two 64-partition groups: they sit in
                        # different PE quadrants, so one group's weight load
                        # can overlap the other group's matmul
                        for ti in range(9):
                            dh, dw = ti // 3, ti % 3
                            nc.tensor.matmul(
                                ps0[:, 0:n],
                                w2t_sb[0:64, ti * 128:(ti + 1) * 128],
                                fqv0[:, dh + r0:dh + r0 + nr, dw:dw + 32],
                                start=(ti == 0), stop=(ti == 8))
                            nc.tensor.matmul(
                                ps1[:, 0:n],
                                w2t_sb[64:128, ti * 128:(ti + 1) * 128],
                                fqv1[:, dh + r0:dh + r0 + nr, dw:dw + 32],
                                start=(ti == 0), stop=(ti == 8))
                        for g2, (ps, p2f) in enumerate(((ps0, p2f0), (ps1, p2f1))):
                            pv = ps[:, 0:n].rearrange(
                                "p (ro t1 wo t2) -> p ro wo t1 t2",
                                t1=2, wo=16, t2=2)
                            nc.vector.tensor_reduce(
                                p2f[:, (r0 // 2) * 16:((r0 + nr) // 2) * 16]
                                .rearrange("p (ro wo) -> p ro wo", wo=16),
                                pv, AX.XY, AL.max)
                    for g2, p2f in enumerate((p2f0, p2f1)):
                        g = c * 2 + g2
                        p2a = p2.tile([128, 400], F32, tag="p2a")
                        nc.scalar.activation(p2a[:], p2f[:], AF.Relu,
                                             bias=bn2b_sb,
                                             scale=bn2s_sb)
                        for bq in range(4):
                            eng = nc.sync if bq % 2 == 0 else nc.scalar
                            eng.dma_start(
                                featsb[g * 4 + bq:g * 4 + bq + 1, :].rearrange(
                                    "p (c q) -> p c q", q=400),
                                p2a[bq * 32:(bq + 1) * 32, :])

            # ---- fc1 GEMM: K=12800; lhsT tiles via PE transpose of featsb ----
            with tc.tile_pool(name="phE", bufs=3) as p4:
                psf = psB.tile([64, 256], F32, tag="b")
                for kk in range(13):
                    nt = 8 if kk < 12 else 4
                    wt8 = p4.tile([128, 8 * 256], F32R, tag="wt8")
                    eng = nc.sync if kk % 2 == 0 else nc.scalar
                    eng.dma_start(
                        wt8[:, 0:nt * 256],
                        rap(w1t, kk * 8 * 32768,
                            [[256, 128], [32768, nt], [1, 256]], F32R))
                    for j in range(nt):
                        k = kk * 8 + j
                        ftp = psB.tile([128, 64], F32, tag="ftp", bufs=2)
                        nc.tensor.transpose(
                            ftp[:], featsb[:, k * 128:(k + 1) * 128], id_sb[:])
                        ft = p4.tile([128, 64], F32R, tag="ft", bufs=4)
                        nc.vector.tensor_copy(ft[:], ftp[:])
                        nc.tensor.matmul(psf[:], ft[:],
                                         wt8[:, j * 256:(j + 1) * 256],
                                         start=(k == 0), stop=(k == 99))
                cur1 = p4.tile([64, 256], F32, tag="cur1")
                nc.scalar.copy(cur1[:], psf[:])
                for h in range(2):
                    pst = psB.tile([128, 64], F32, tag="b")
                    nc.tensor.transpose(pst[:], cur1[:, h * 128:(h + 1) * 128],
                                        id_sb[:])
                    nc.vector.tensor_scalar(cur1T[:, h * 64:(h + 1) * 64],
                                            pst[:], b1_sb[:, h:h + 1], None,
                                            AL.add)

            if DBG:
                nc.sync.dma_start(dfeat.ap(), featsb[:])
                nc.sync.dma_start(dcur.ap(), cur1T[:])
                nc.sync.dma_start(dxmp.ap(), xmp.ap())

            # ---- LIF scan (reset_t == spike_{t-1}; biases on scalar engine) ----
            with tc.tile_pool(name="phF", bufs=3) as p5, \
                 tc.tile_pool(name="phG", bufs=1) as p6:
                outsb = p6.tile([35, T * 64], F32, tag="outsb")
                m1 = p6.tile([128, 128], F32, tag="m1")
                m2 = p6.tile([128, 64], F32, tag="m2")
                m3 = p6.tile([35, 64], F32, tag="m3")
                s1z = p6.tile([128, 128], F32R, tag="s1z")
                s2z = p6.tile([128, 64], F32R, tag="s2z")
                nc.gpsimd.memset(m1[:], 0.0)
                nc.gpsimd.memset(m2[:], 0.0)
                nc.gpsimd.memset(m3[:], 0.0)
                nc.gpsimd.memset(s1z[:].bitcast(F32), 0.0)
                nc.gpsimd.memset(s2z[:].bitcast(F32), 0.0)
                s1_prev, s2_prev = s1z, s2z
                for t in range(T):
                    # layer 1: m1 = be1*m1 + (cur1T - s1_prev); s1 = m1>1
                    t1 = p5.tile([128, 128], F32, tag="t1")
                    nc.vector.scalar_tensor_tensor(
                        t1[:], s1_prev[:].bitcast(F32), -1.0, cur1T[:],
                        AL.mult, AL.add)
                    nc.vector.scalar_tensor_tensor(
                        m1[:], m1[:], be1, t1[:], AL.mult, AL.add)
                    s1 = p5.tile([128, 128], F32R, tag="s1")
                    nc.vector.tensor_scalar(s1[:], m1[:], 1.0, None, AL.is_gt)
                    ps2 = psB.tile([128, 64], F32, tag="b")
                    nc.tensor.matmul(ps2[:], w2a_sb[:],
                                     s1[:, 0:64],
                                     start=True, stop=False)
                    nc.tensor.matmul(ps2[:], w2b_sb[:],
                                     s1[:, 64:128],
                                     start=False, stop=True)
                    # layer 2: in2 = ps2 + b2 - s2_prev (scalar engine adds bias)
                    t2 = p5.tile([128, 64], F32, tag="t2")
                    nc.scalar.activation(t2[:], ps2[:], AF.Identity,
                                         bias=b2_sb, scale=1.0)
                    nc.vector.scalar_tensor_tensor(
                        t2[:], s2_prev[:].bitcast(F32), -1.0, t2[:],
                        AL.mult, AL.add)
                    nc.vector.scalar_tensor_tensor(
                        m2[:], m2[:], be2, t2[:], AL.mult, AL.add)
                    s2 = p5.tile([128, 64], F32R, tag="s2")
                    nc.vector.tensor_scalar(s2[:], m2[:], 1.0, None, AL.is_gt)
                    ps3 = psB.tile([35, 64], F32, tag="b")
                    nc.tensor.matmul(ps3[:], w3t_sb[:],
                                     s2[:],
                                     start=True, stop=True)
                    # layer 3: m3 = be3*m3 + (ps3 + b3 - s3_prev)
                    t3 = p5.tile([35, 64], F32, tag="t3")
                    nc.scalar.activation(t3[:], ps3[:], AF.Identity,
                                         bias=b3_sb, scale=1.0)
                    s3prev = outsb[:, (t - 1) * 64:t * 64] if t > 0 else outsb[:, 0:64]
                    if t > 0:
                        nc.vector.tensor_sub(t3[:], t3[:], s3prev)
                    nc.vector.scalar_tensor_tensor(
                        m3[:], m3[:], be3, t3[:], AL.mult, AL.add)
                    nc.vector.tensor_scalar(outsb[:, t * 64:(t + 1) * 64],
                                            m3[:], 1.0, None, AL.is_gt)
                    s1_prev, s2_prev = s1, s2
                # transpose spikes to (t,b)-major so the out DMA writes
                # 140B-contiguous lines instead of a 4-byte scatter
                osb2 = p6.tile([128, 13 * 35], F32, tag="osb2")
                for cchunk in range(13):
                    n = 128 if cchunk < 12 else 64
                    pso = psB.tile([128, 35], F32, tag="ftp", bufs=2)
                    nc.tensor.transpose(
                        pso[0:n, :], outsb[:, cchunk * 128:cchunk * 128 + n],
                        id_sb[0:35, 0:35])
                    nc.vector.tensor_copy(
                        osb2[0:n, cchunk * 35:(cchunk + 1) * 35], pso[0:n, :])
                nc.sync.dma_start(
                    rap(out, 0, [[35, 128], [4480, 12], [1, 35]]),
                    osb2[:, 0:12 * 35].rearrange("p (c q) -> p c q", q=35))
                nc.sync.dma_start(
                    rap(out, 12 * 4480, [[35, 64], [1, 35]]),
                    osb2[0:64, 12 * 35:13 * 35])

    split_multi_waits(nc)
    return nc


def prep(inputs):
    f = np.float32
    w1 = np.asarray(inputs["conv1_w"], f)
    s1v = np.asarray(inputs["bn1_g"], f) / np.sqrt(
        np.asarray(inputs["bn1_v"], f) + 1e-5)
    sh1 = np.asarray(inputs["bn1_b"], f) + (
        np.asarray(inputs["conv1_b"], f) - np.asarray(inputs["bn1_m"], f)) * s1v
    w2 = np.asarray(inputs["conv2_w"], f)
    s2v = np.asarray(inputs["bn2_g"], f) / np.sqrt(
        np.asarray(inputs["bn2_v"], f) + 1e-5)
    sh2 = np.asarray(inputs["bn2_b"], f) + (
        np.asarray(inputs["conv2_b"], f) - np.asarray(inputs["bn2_m"], f)) * s2v

    w1b = np.zeros((72, 128), f)
    for bg in range(8):
        for ch in range(16):
            for dh in range(3):
                for dw in range(3):
                    w1b[dh * 24 + dw * 8 + bg, bg * 16 + ch] = \
                        w1[ch, 0, dh, dw] / 25.0
    bn1sv = np.tile(s1v, 8).astype(f)
    bn1bv = np.tile(sh1, 8).astype(f)

    w2t9 = np.zeros((9, 64, 128), f)
    for ti in range(9):
        dh, dw = ti // 3, ti % 3
        for bg in range(4):
            for ci in range(16):
                for co in range(32):
                    w2t9[ti, bg * 16 + ci, bg * 32 + co] = w2[co, ci, dh, dw]
    bn2sv = np.tile(s2v, 4).astype(f)
    bn2bv = np.tile(sh2, 4).astype(f)

    b1v = np.asarray(inputs["fc1_b"], f)
    b2v = np.asarray(inputs["fc2_b"], f)
    b3v = np.asarray(inputs["fc3_b"], f)
    cst = np.zeros((128, 8), f)
    cst[:, 0] = bn1sv
    cst[:, 1] = bn1bv
    cst[:, 2] = bn2sv
    cst[:, 3] = bn2bv
    cst[:, 4] = b1v[0:128]
    cst[:, 5] = b1v[128:256]
    cst[:, 6] = b2v
    cst[0:35, 7] = b3v

    return dict(
        w1b=w1b, w2t=w2t9, cst=cst,
        w1t=np.ascontiguousarray(np.asarray(inputs["fc1_w"], f).T),
        w2a=np.ascontiguousarray(np.asarray(inputs["fc2_w"], f).T[0:128]),
        w2b=np.ascontiguousarray(np.asarray(inputs["fc2_w"], f).T[128:256]),
        w3t=np.ascontiguousarray(np.asarray(inputs["fc3_w"], f).T),
        ident=np.eye(64, dtype=f),
    )


def kernel(**inputs):
    f = np.float32
    x = np.asarray(inputs["x"], f)
    be1 = float(np.clip(np.asarray(inputs["beta1"], f), 0.0, 1.0))
    be2 = float(np.clip(np.asarray(inputs["beta2"], f), 0.0, 1.0))
    be3 = float(np.clip(np.asarray(inputs["beta3"], f), 0.0, 1.0))
    consts = prep(inputs)
    nc = build(be1, be2, be3)
    in_maps = []
    for c in range(NCORE):
        m = {"x": np.ascontiguousarray(x[:, c * BC:(c + 1) * BC])}
        m.update(consts)
        in_maps.append(m)
    res = bass_utils.run_bass_kernel_spmd(nc, in_maps, core_ids=list(range(NCORE)))
    return np.concatenate([res.results[c]["out"] for c in range(NCORE)], axis=1)


# revision 36
# speedup vs baseline: 2.3232x; 1.2159x over previous
"""KWS-SNN Trainium2 kernel: 8-way batch-parallel Bass/Tile implementation.

Per core (BC=64): mean over T (DMA-fed vector adds) -> padded image to DRAM
-> conv1 (block-diag batch-packed, K=72, fp32r) with single-op 2x2 PSUM
pool-reduce -> BN+ReLU into SBUF-resident padded f1 -> conv2 (9 tap-matmuls,
K=64, fp32r) pool-reduce -> BN+ReLU -> featd round trip -> fc1 GEMM against
SBUF-preloaded weights (fp32r) -> transpose -> 25-step LIF scan -> spikes.
DMA is spread across the two HWDGE queues (sync/scalar) + gpsimd SWDGE.
"""
import sys
sys.path.insert(0, '/opt/trn_rl_repo')
import numpy as np
import concourse.bass as bass
import concourse.mybir as mybir
import bass_rust
from concourse.tile import TileContext
from concourse import bass_utils

F32 = mybir.dt.float32
F32R = mybir.dt.float32r
AL = mybir.AluOpType
AF = mybir.ActivationFunctionType
AX = mybir.AxisListType

T, BF, H, W = 25, 512, 100, 64
NCORE = 8
BC = BF // NCORE          # 64 batches per core
NCL = 35

XMP_B = 102 * 66          # 6732 per-batch padded mean image


def rap(handle, off, dims, dt=None):
    a = handle.ap()
    if dt is not None:
        a = a.bitcast(dt)
    return bass_rust.AP(tensor=a.tensor, offset=off, ap=[list(d) for d in dims])


def split_multi_waits(nc, max_waits=1):
    """This walrus build rejects >1 sync-wait per instruction; hoist excess
    waits onto same-engine NoOps inserted immediately before."""
    ctr = 0
    for f in nc.m.functions:
        for bb in f.blocks:
            il = bb.instructions
            if not any(i.sync_info is not None and len(i.sync_info.on_wait) > max_waits
                       for i in il):
                continue
            new = []
            for inst in il:
                si = inst.sync_info
                if si is not None and len(si.on_wait) > max_waits:
                    waits = list(si.on_wait)
                    while len(waits) > max_waits:
                        w = waits.pop(0)
                        nop = mybir.InstNoOp(
                            name=f"_ws_{ctr}", engine=inst.engine,
                            sync_info=mybir.SyncInfo(on_wait=[w], on_update=[]),
                            bass_nofuse=True)
                        ctr += 1
                        new.append(nop)
                    inst.sync_info = mybir.SyncInfo(
                        on_wait=waits, on_update=list(si.on_update))
                new.append(inst)
            bb.instructions = new
    return ctr


def build(be1, be2, be3):
    nc = bass.Bass()
    xin = nc.dram_tensor("x", [T, BC, H, W], F32, kind="ExternalInput")
    w1b = nc.dram_tensor("w1b", [72, 128], F32, kind="ExternalInput")
    w2t = nc.dram_tensor("w2t", [9, 64, 128], F32, kind="ExternalInput")
    w1t = nc.dram_tensor("w1t", [12800, 256], F32, kind="ExternalInput")
    cst = nc.dram_tensor("cst", [128, 8], F32, kind="ExternalInput")
    w2a = nc.dram_tensor("w2a", [128, 128], F32, kind="ExternalInput")
    w2b = nc.dram_tensor("w2b", [128, 128], F32, kind="ExternalInput")
    w3t = nc.dram_tensor("w3t", [128, 35], F32, kind="ExternalInput")
    ident = nc.dram_tensor("ident", [64, 64], F32, kind="ExternalInput")

    xmp = nc.dram_tensor("xmp", [BC * XMP_B], F32, kind="Internal")
    xm = nc.dram_tensor("xm", [BC * 6400], F32, kind="Internal")
    out = nc.dram_tensor("out", [T, BC, NCL], F32, kind="ExternalOutput")
    DBG = bool(__import__("os").environ.get("SNN_DBG"))
    if DBG:
        dfeat = nc.dram_tensor("dfeat", [BC, 12800], F32, kind="ExternalOutput")
        dcur = nc.dram_tensor("dcur", [128, 128], F32, kind="ExternalOutput")
        dxmp = nc.dram_tensor("dxmp", [BC * XMP_B], F32, kind="ExternalOutput")
        dhp = nc.dram_tensor("dhp", [128, 1600], F32, kind="ExternalOutput")
        df1 = nc.dram_tensor("df1", [128, 1768], F32, kind="ExternalOutput")

    with TileContext(nc) as tc:
        with (
            tc.tile_pool(name="const", bufs=1) as pc,
            tc.tile_pool(name="main", bufs=1) as pm,
            tc.tile_pool(name="psA", bufs=3, space="PSUM") as psA,
            tc.tile_pool(name="psB", bufs=3, space="PSUM") as psB,
        ):
            # ---- constants to SBUF ----
            w1b_sb = pc.tile([72, 128], F32R, tag="w1b")
            nc.sync.dma_start(w1b_sb[:], w1b.ap().bitcast(F32R))
            # conv2 weights duplicated into both partition halves so either
            # f1pad 64-slice can be the matmul rhs (equal base-partition rule)
            w2t_sb = pc.tile([128, 9 * 128], F32R, tag="w2t")
            nc.sync.dma_start(
                w2t_sb[0:64, :], rap(w2t, 0, [[128, 64], [8192, 9], [1, 128]], F32R))
            nc.sync.dma_start(
                w2t_sb[64:128, :], rap(w2t, 0, [[128, 64], [8192, 9], [1, 128]], F32R))
            # packed per-partition constants: cols = bn1s bn1b bn2s bn2b b1h0 b1h1 b2 b3
            cst_sb = pc.tile([128, 8], F32, tag="cst")
            nc.scalar.dma_start(cst_sb[:], cst.ap())
            bn1s_sb, bn1b_sb = cst_sb[:, 0:1], cst_sb[:, 1:2]
            bn2s_sb, bn2b_sb = cst_sb[:, 2:3], cst_sb[:, 3:4]
            b1_sb = cst_sb[:, 4:6]
            b2_sb = cst_sb[:, 6:7]
            b3_sb = cst_sb[0:35, 7:8]
            w2a_sb = pc.tile([128, 128], F32R, tag="w2a")
            nc.scalar.dma_start(w2a_sb[:], w2a.ap().bitcast(F32R))
            w2b_sb = pc.tile([128, 128], F32R, tag="w2b")
            nc.scalar.dma_start(w2b_sb[:], w2b.ap().bitcast(F32R))
            w3t_sb = pc.tile([128, 35], F32R, tag="w3t")
            nc.scalar.dma_start(w3t_sb[:], w3t.ap().bitcast(F32R))
            id_sb = pc.tile([64, 64], F32, tag="id")
            nc.scalar.dma_start(id_sb[:], ident.ap())

            cur1T = pm.tile([128, 128], F32, tag="cur1T")
            featsb = pm.tile([64, 12800], F32, tag="featsb")

            # ---- phase A: sum over T (mean /25 folded into conv1 w) ----
            with (
                tc.tile_pool(name="phZ", bufs=1) as pz,
                tc.tile_pool(name="phA", bufs=4) as pa,
            ):
                # zero-fill padded image surface early
                zt = pz.tile([128, 3366], F32, tag="zt")
                nc.gpsimd.memset(zt[:], 0.0)
                nc.sync.dma_start(
                    rap(xmp, 0, [[3366, 128], [1, 3366]]), zt[:])
                acc = pz.tile([128, 3200], F32, tag="acc")
                for t in range(T):
                    xt = pa.tile([128, 3200], F32, tag="xt")
                    eng = nc.sync if t % 2 == 0 else nc.scalar
                    eng.dma_start(
                        xt[:], rap(xin, t * 409600, [[3200, 128], [1, 3200]]))
                    if t == 0:
                        nc.vector.tensor_copy(acc[:], xt[:])
                    else:
                        nc.vector.tensor_add(acc[:], acc[:], xt[:])
                # dump sum flat, then pad via DRAM->DRAM scatter
                nc.sync.dma_start(
                    rap(xm, 0, [[3200, 128], [1, 3200]]), acc[:])
                nc.sync.dma_start(
                    rap(xmp, 67, [[XMP_B, 8], [66, 100], [1, 64]]),
                    rap(xm, 0, [[1, 8 * 6400]]))
                nc.scalar.dma_start(
                    rap(xmp, 8 * XMP_B + 67, [[XMP_B, 56], [66, 100], [1, 64]]),
                    rap(xm, 8 * 6400, [[1, 56 * 6400]]))

            # ---- conv1 + conv2, pipelined per 8-batch chunk ----
            with (
                tc.tile_pool(name="phC", bufs=2) as p1,
                tc.tile_pool(name="phD", bufs=2) as p2,
                tc.tile_pool(name="phF1", bufs=2) as pf,
            ):
                rblk = [(0, 14), (14, 12), (26, 12), (38, 12)]
                for c in range(8):
                    im1 = p1.tile([72, 6400], F32R, tag="im1")
                    for dh in range(3):
                        for dw in range(3):
                            # SWDGE queue: keeps im1 prefetch out of the
                            # hwdge queues where featsb writes (gated on
                            # conv2 completion) would head-of-line block it
                            nc.gpsimd.dma_start(
                                im1[dh * 24 + dw * 8:dh * 24 + dw * 8 + 8, :],
                                rap(xmp, c * 8 * XMP_B + dh * 66 + dw,
                                    [[XMP_B, 8], [66, 100], [1, 64]], F32R))
                    f1pad = pf.tile([128, 1768], F32R, tag="f1pad")
                    nc.gpsimd.memset(f1pad[:].bitcast(F32), 0.0)
                    hp = p1.tile([128, 1600], F32, tag="hp")
                    for s in range(13):
                        n = 512 if s < 12 else 256
                        rows = 8 if s < 12 else 4
                        ps = psA.tile([128, 512], F32, tag="cv")
                        nc.tensor.matmul(
                            ps[:, 0:n], w1b_sb[:],
                            im1[:, s * 512:s * 512 + n],
                            start=True, stop=True)
                        pv = ps[:, 0:n].rearrange(
                            "p (ro t1 wo t2) -> p ro wo t1 t2",
                            t1=2, wo=32, t2=2)
                        nc.vector.tensor_reduce(
                            hp[:, s * 128:s * 128 + rows * 16].rearrange(
                                "p (ro wo) -> p ro wo", wo=32),
                            pv, AX.XY, AL.max)
                    fv = f1pad[:].rearrange("p (r w) -> p r w", w=34)
                    nc.scalar.activation(
                        fv[:, 1:51, 1:33],
                        hp[:].rearrange("p (r w) -> p r w", w=32),
                        AF.Relu, bias=bn1b_sb, scale=bn1s_sb)
                    if DBG and c == 7:
                        nc.sync.dma_start(dhp.ap(), hp[:])
                        nc.sync.dma_start(df1.ap(), f1pad[:].bitcast(F32))

                    fqv0 = f1pad[0:64, :].rearrange("p (r w) -> p r w", w=34)
                    fqv1 = f1pad[64:128, :].rearrange("p (r w) -> p r w", w=34)
                    p2f0 = p2.tile([128, 400], F32, tag="p2f0")
                    p2f1 = p2.tile([128, 400], F32, tag="p2f1")
                    for (r0, nr) in rblk:
                        ps0 = psA.tile([128, 512], F32, tag="cv")
                        ps1 = psA.tile([128, 512], F32, tag="cv")
                        n = nr * 32
                        # interleave the two 64-partition groups: they sit in
                        # different PE quadrants, so one group's weight load
                        # can overlap the other group's matmul
                        for ti in range(9):
                            dh, dw = ti // 3, ti % 3
                            nc.tensor.matmul(
                                ps0[:, 0:n],
                                w2t_sb[0:64, ti * 128:(ti + 1) * 128],
                                fqv0[:, dh + r0:dh + r0 + nr, dw:dw + 32],
                                start=(ti == 0), stop=(ti == 8))
                            nc.tensor.matmul(
                                ps1[:, 0:n],
                                w2t_sb[64:128, ti * 128:(ti + 1) * 128],
                                fqv1[:, dh + r0:dh + r0 + nr, dw:dw + 32],
                                start=(ti == 0), stop=(ti == 8))
                        for g2, (ps, p2f) in enumerate(((ps0, p2f0), (ps1, p2f1))):
                            pv = ps[:, 0:n].rearrange(
                                "p (ro t1 wo t2) -> p ro wo t1 t2",
                                t1=2, wo=16, t2=2)
                            nc.vector.tensor_reduce(
                                p2f[:, (r0 // 2) * 16:((r0 + nr) // 2) * 16]
                                .rearrange("p (ro wo) -> p ro wo", wo=16),
                                pv, AX.XY, AL.max)
                    for g2, p2f in enumerate((p2f0, p2f1)):
                        g = c * 2 + g2
                        p2a = p2.tile([128, 400], F32, tag="p2a")
                        nc.scalar.activation(p2a[:], p2f[:], AF.Relu,
                                             bias=bn2b_sb,
                                             scale=bn2s_sb)
                        for bq in range(4):
                            eng = nc.sync if bq % 2 == 0 else nc.scalar
                            eng.dma_start(
                                featsb[g * 4 + bq:g * 4 + bq + 1, :].rearrange(
                                    "p (c q) -> p c q", q=400),
                                p2a[bq * 32:(bq + 1) * 32, :])

            # ---- fc1 GEMM: K=12800; lhsT tiles via PE transpose of featsb ----
            with tc.tile_pool(name="phE", bufs=3) as p4:
                psf = psB.tile([64, 256], F32, tag="b")
                for kk in range(13):
                    nt = 8 if kk < 12 else 4
                    wt8 = p4.tile([128, 8 * 256], F32R, tag="wt8")
                    eng = nc.sync if kk % 2 == 0 else nc.scalar
                    eng.dma_start(
                        wt8[:, 0:nt * 256],
                        rap(w1t, kk * 8 * 32768,
                            [[256, 128], [32768, nt], [1, 256]], F32R))
                    for j in range(nt):
                        k = kk * 8 + j
                        ftp = psB.tile([128, 64], F32, tag="ftp", bufs=2)
                        nc.tensor.transpose(
                            ftp[:], featsb[:, k * 128:(k + 1) * 128], id_sb[:])
                        ft = p4.tile([128, 64], F32R, tag="ft", bufs=4)
                        nc.vector.tensor_copy(ft[:], ftp[:])
                        nc.tensor.matmul(psf[:], ft[:],
                                         wt8[:, j * 256:(j + 1) * 256],
                                         start=(k == 0), stop=(k == 99))
                cur1 = p4.tile([64, 256], F32, tag="cur1")
                nc.scalar.copy(cur1[:], psf[:])
                for h in range(2):
                    pst = psB.tile([128, 64], F32, tag="b")
                    nc.tensor.transpose(pst[:], cur1[:, h * 128:(h + 1) * 128],
                                        id_sb[:])
                    nc.vector.tensor_scalar(cur1T[:, h * 64:(h + 1) * 64],
                                            pst[:], b1_sb[:, h:h + 1], None,
                                            AL.add)

            if DBG:
                nc.sync.dma_start(dfeat.ap(), featsb[:])
                nc.sync.dma_start(dcur.ap(), cur1T[:])
                nc.sync.dma_start(dxmp.ap(), xmp.ap())

            # ---- LIF scan (reset_t == spike_{t-1}; biases on scalar engine) ----
            with tc.tile_pool(name="phF", bufs=3) as p5, \
                 tc.tile_pool(name="phG", bufs=1) as p6:
                outsb = p6.tile([35, T * 64], F32, tag="outsb")
                m1 = p6.tile([128, 128], F32, tag="m1")
                m2 = p6.tile([128, 64], F32, tag="m2")
                m3 = p6.tile([35, 64], F32, tag="m3")
                s1z = p6.tile([128, 128], F32R, tag="s1z")
                s2z = p6.tile([128, 64], F32R, tag="s2z")
                nc.gpsimd.memset(m1[:], 0.0)
                nc.gpsimd.memset(m2[:], 0.0)
                nc.gpsimd.memset(m3[:], 0.0)
                nc.gpsimd.memset(s1z[:].bitcast(F32), 0.0)
                nc.gpsimd.memset(s2z[:].bitcast(F32), 0.0)
                s1_prev, s2_prev = s1z, s2z
                for t in range(T):
                    # layer 1: m1 = be1*m1 + (cur1T - s1_prev); s1 = m1>1
                    t1 = p5.tile([128, 128], F32, tag="t1")
                    nc.vector.scalar_tensor_tensor(
                        t1[:], s1_prev[:].bitcast(F32), -1.0, cur1T[:],
                        AL.mult, AL.add)
                    nc.vector.scalar_tensor_tensor(
                        m1[:], m1[:], be1, t1[:], AL.mult, AL.add)
                    s1 = p5.tile([128, 128], F32R, tag="s1")
                    nc.vector.tensor_scalar(s1[:], m1[:], 1.0, None, AL.is_gt)
                    ps2 = psB.tile([128, 64], F32, tag="b")
                    nc.tensor.matmul(ps2[:], w2a_sb[:],
                                     s1[:, 0:64],
                                     start=True, stop=False)
                    nc.tensor.matmul(ps2[:], w2b_sb[:],
                                     s1[:, 64:128],
                                     start=False, stop=True)
                    # layer 2: in2 = ps2 + b2 - s2_prev (scalar engine adds bias)
                    t2 = p5.tile([128, 64], F32, tag="t2")
                    nc.scalar.activation(t2[:], ps2[:], AF.Identity,
                                         bias=b2_sb, scale=1.0)
                    nc.vector.scalar_tensor_tensor(
                        t2[:], s2_prev[:].bitcast(F32), -1.0, t2[:],
                        AL.mult, AL.add)
                    nc.vector.scalar_tensor_tensor(
                        m2[:], m2[:], be2, t2[:], AL.mult, AL.add)
                    s2 = p5.tile([128, 64], F32R, tag="s2")
                    nc.vector.tensor_scalar(s2[:], m2[:], 1.0, None, AL.is_gt)
                    ps3 = psB.tile([35, 64], F32, tag="b")
                    nc.tensor.matmul(ps3[:], w3t_sb[:],
                                     s2[:],
                                     start=True, stop=True)
                    # layer 3: m3 = be3*m3 + (ps3 + b3 - s3_prev)
                    t3 = p5.tile([35, 64], F32, tag="t3")
                    nc.scalar.activation(t3[:], ps3[:], AF.Identity,
                                         bias=b3_sb, scale=1.0)
                    s3prev = outsb[:, (t - 1) * 64:t * 64] if t > 0 else outsb[:, 0:64]
                    if t > 0:
                        nc.vector.tensor_sub(t3[:], t3[:], s3prev)
                    nc.vector.scalar_tensor_tensor(
                        m3[:], m3[:], be3, t3[:], AL.mult, AL.add)
                    nc.vector.tensor_scalar(outsb[:, t * 64:(t + 1) * 64],
                                            m3[:], 1.0, None, AL.is_gt)
                    s1_prev, s2_prev = s1, s2
                # transpose spikes to (t,b)-major so the out DMA writes
                # 140B-contiguous lines instead of a 4-byte scatter
                osb2 = p6.tile([128, 13 * 35], F32, tag="osb2")
                for cchunk in range(13):
                    n = 128 if cchunk < 12 else 64
                    pso = psB.tile([128, 35], F32, tag="ftp", bufs=2)
                    nc.tensor.transpose(
                        pso[0:n, :], outsb[:, cchunk * 128:cchunk * 128 + n],
                        id_sb[0:35, 0:35])
                    nc.vector.tensor_copy(
                        osb2[0:n, cchunk * 35:(cchunk + 1) * 35], pso[0:n, :])
                nc.sync.dma_start(
                    rap(out, 0, [[35, 128], [4480, 12], [1, 35]]),
                    osb2[:, 0:12 * 35].rearrange("p (c q) -> p c q", q=35))
                nc.sync.dma_start(
                    rap(out, 12 * 4480, [[35, 64], [1, 35]]),
                    osb2[0:64, 12 * 35:13 * 35])

    split_multi_waits(nc)
    return nc


def prep(inputs):
    f = np.float32
    w1 = np.asarray(inputs["conv1_w"], f)
    s1v = np.asarray(inputs["bn1_g"], f) / np.sqrt(
        np.asarray(inputs["bn1_v"], f) + 1e-5)
    sh1 = np.asarray(inputs["bn1_b"], f) + (
        np.asarray(inputs["conv1_b"], f) - np.asarray(inputs["bn1_m"], f)) * s1v
    w2 = np.asarray(inputs["conv2_w"], f)
    s2v = np.asarray(inputs["bn2_g"], f) / np.sqrt(
        np.asarray(inputs["bn2_v"], f) + 1e-5)
    sh2 = np.asarray(inputs["bn2_b"], f) + (
        np.asarray(inputs["conv2_b"], f) - np.asarray(inputs["bn2_m"], f)) * s2v

    w1b = np.zeros((72, 128), f)
    for bg in range(8):
        for ch in range(16):
            for dh in range(3):
                for dw in range(3):
                    w1b[dh * 24 + dw * 8 + bg, bg * 16 + ch] = \
                        w1[ch, 0, dh, dw] / 25.0
    bn1sv = np.tile(s1v, 8).astype(f)
    bn1bv = np.tile(sh1, 8).astype(f)

    w2t9 = np.zeros((9, 64, 128), f)
    for ti in range(9):
        dh, dw = ti // 3, ti % 3
        for bg in range(4):
            for ci in range(16):
                for co in range(32):
                    w2t9[ti, bg * 16 + ci, bg * 32 + co] = w2[co, ci, dh, dw]
    bn2sv = np.tile(s2v, 4).astype(f)
    bn2bv = np.tile(sh2, 4).astype(f)

    b1v = np.asarray(inputs["fc1_b"], f)
    b2v = np.asarray(inputs["fc2_b"], f)
    b3v = np.asarray(inputs["fc3_b"], f)
    cst = np.zeros((128, 8), f)
    cst[:, 0] = bn1sv
    cst[:, 1] = bn1bv
    cst[:, 2] = bn2sv
    cst[:, 3] = bn2bv
    cst[:, 4] = b1v[0:128]
    cst[:, 5] = b1v[128:256]
    cst[:, 6] = b2v
    cst[0:35, 7] = b3v

    return dict(
        w1b=w1b, w2t=w2t9, cst=cst,
        w1t=np.ascontiguousarray(np.asarray(inputs["fc1_w"], f).T),
        w2a=np.ascontiguousarray(np.asarray(inputs["fc2_w"], f).T[0:128]),
        w2b=np.ascontiguousarray(np.asarray(inputs["fc2_w"], f).T[128:256]),
        w3t=np.ascontiguousarray(np.asarray(inputs["fc3_w"], f).T),
        ident=np.eye(64, dtype=f),
    )


def kernel(**inputs):
    f = np.float32
    x = np.asarray(inputs["x"], f)
    be1 = float(np.clip(np.asarray(inputs["beta1"], f), 0.0, 1.0))
    be2 = float(np.clip(np.asarray(inputs["beta2"], f), 0.0, 1.0))
    be3 = float(np.clip(np.asarray(inputs["beta3"], f), 0.0, 1.0))
    consts = prep(inputs)
    nc = build(be1, be2, be3)
    in_maps = []
    for c in range(NCORE):
        m = {"x": np.ascontiguousarray(x[:, c * BC:(c + 1) * BC])}
        m.update(consts)
        in_maps.append(m)
    res = bass_utils.run_bass_kernel_spmd(nc, in_maps, core_ids=list(range(NCORE)))
    return np.concatenate([res.results[c]["out"] for c in range(NCORE)], axis=1)
